# revision 12
# baseline (speedup 1.0000x reference)
"""Trainium2 Bass kernel for nn_Net_67954972557347 (dense_mlp).

Network: a1 = lrelu(a@Wa+ba) [B,68]; b1 = lrelu(b@Wb+bb) [B,68];
c = [a1|b1|meta] [B,140]; then 10 lrelu'd dense layers
(140->34->34->20->20->20->20->20->5->2->1), lrelu slope 0.01.

Strategy: pure data parallel over 8 cores (32768 rows each), activations
feature-major ([feat, batch]), batch streamed 512 columns per chunk.
All tensors fp16 (same 10-bit mantissa as tf32; fp32 PSUM accumulation).

12 layers in SIX matmuls per chunk via a software pipeline (one
layer-hop per step; stages of different chunks colocate). DMA'd tiles
are full 128-partition (partial-partition DMAs run ~10x slower), state
tiles are drain-only:

  TA [a(45); ones; pad->128]   (quad, full-width DMA)
  TB [b(102); ones; ilrelu(meta)(4); pad->128]  (quad, full-width DMA)
  S  [c0(34); c2(20); c6(20); c8(2); c4(20); one(1)]  (per step, drained)
  U[:, 0:512]=TC [a1; c1; c3; c7; y];  U[:, 512:1024]=TD [b1; c5; meta; one]

  G1i: TA[0:46]  -> Qab bank0 (a1+ba)              start
  G1s: S[0:97]   -> Qab bank0 (c1,c3,c7,y + biases) stop
  G2 : TB[0:107] -> Qab bank1 (b1+bb, meta-pass, one-pass) start
  G5 : S[0:97]   -> Qab bank1 (c5+B5)               stop
  G3 : U-TC      -> Q34 (c0p, c2, c4, c8)           start
  G4 : U-TD      -> Q34 (c0p, c6, one-pass, even biases) stop

Drains (2 per chunk): Qab[0:128, 0:1024] -> U' (one ScalarE Prelu over
2 PSUM banks); Q34[0:97] -> S' (VectorE cast + stt lrelu). The ones row
self-sustains: TB ones -> U-TD one -> Q34 col 96 -> S one. meta rides
G2 as identity passthrough (host pre-applies inverse-lrelu). y is DMA'd
from U row 127 on the gpsimd queue.
"""

import os
import sys

import numpy as np

for _p in ("/opt/trn_rl_repo", "/root/.axon_site/_ro/trn_rl_repo"):
    if os.path.isdir(_p) and _p not in sys.path:
        sys.path.append(_p)

import concourse.bass as bass
import concourse.mybir as mybir
import concourse.tile as tile
from concourse import bacc
from concourse.bass_utils import run_bass_kernel_spmd
from bass_rust import add_dep_helper

F16 = mybir.dt.float16
F32 = mybir.dt.float32
ALU = mybir.AluOpType
PRELU = mybir.ActivationFunctionType.Prelu

B_FULL = 262144
N_CORES = 8
B_CORE = B_FULL // N_CORES          # 32768
N = 512                              # columns per chunk (PSUM fp32 cap)
PIPE = 20                            # pipeline depth (2 steps per layer hop)
ALPHA = 0.01
QUAD = 4                             # chunks per input tile / DMA

# TA rows
TA_A, TA_ONE, TA_END = 0, 45, 46
# TB rows
TB_B, TB_ONE, TB_M, TB_END = 0, 102, 103, 107
# S rows (== Q34 column layout)
S_C0, S_C2, S_C6, S_C8, S_C4, S_ONE, S_END = 0, 34, 54, 74, 76, 96, 97
# U cols 0:512 = TC rows (== Qab bank0 cols)
TC_A1, TC_C1, TC_C3, TC_C7, TC_Y, TC_END = 0, 68, 102, 122, 127, 128
# U cols 512:1024 = TD rows (== Qab bank1 cols)
TD_B1, TD_C5, TD_M, TD_ONE, TD_END = 0, 68, 88, 92, 93


def _pack_weights(inp):
    """Six [128,128] fp16 stationaries packed into one [128, 768] tile."""
    f32 = lambda k: np.asarray(inp[k], np.float32)
    Wa, ba, Wb, bb = f32("Wa"), f32("ba"), f32("Wb"), f32("bb")
    W = [f32(f"W{i}") for i in range(10)]
    B = [f32(f"B{i}") for i in range(10)]
    wt = np.zeros((128, 768), np.float32)

    g1i = wt[:, 0:128]                 # TA -> bank0
    g1i[TA_A:TA_ONE, TC_A1:TC_C1] = Wa
    g1i[TA_ONE, TC_A1:TC_C1] = ba

    g1s = wt[:, 128:256]               # S -> bank0
    g1s[S_C0:S_C2, TC_C1:TC_C3] = W[1]
    g1s[S_C2:S_C6, TC_C3:TC_C7] = W[3]
    g1s[S_C6:S_C8, TC_C7:TC_Y] = W[7]
    g1s[S_C8:S_C4, TC_Y:TC_END] = W[9]
    g1s[S_ONE, TC_C1:TC_C3] = B[1]
    g1s[S_ONE, TC_C3:TC_C7] = B[3]
    g1s[S_ONE, TC_C7:TC_Y] = B[7]
    g1s[S_ONE, TC_Y:TC_END] = B[9]

    g2 = wt[:, 256:384]                # TB -> bank1
    g2[TB_B:TB_ONE, TD_B1:TD_C5] = Wb
    g2[TB_ONE, TD_B1:TD_C5] = bb
    g2[TB_ONE, TD_ONE] = 1.0                      # ones passthrough
    g2[TB_M:TB_END, TD_M:TD_ONE] = np.eye(4)      # meta passthrough

    g5 = wt[:, 384:512]                # S -> bank1
    g5[S_C4:S_ONE, TD_C5:TD_M] = W[5]
    g5[S_ONE, TD_C5:TD_M] = B[5]

    g3 = wt[:, 512:640]                # U-TC -> Q34
    g3[TC_A1:TC_C1, S_C0:S_C2] = W[0][0:68]
    g3[TC_C1:TC_C3, S_C2:S_C6] = W[2]
    g3[TC_C3:TC_C7, S_C4:S_ONE] = W[4]
    g3[TC_C7:TC_Y, S_C8:S_C4] = W[8]

    g4 = wt[:, 640:768]                # U-TD -> Q34
    g4[TD_B1:TD_C5, S_C0:S_C2] = W[0][68:136]
    g4[TD_C5:TD_M, S_C6:S_C8] = W[6]
    g4[TD_M:TD_ONE, S_C0:S_C2] = W[0][136:140]
    g4[TD_ONE, S_C0:S_C2] = B[0]
    g4[TD_ONE, S_C2:S_C6] = B[2]
    g4[TD_ONE, S_C6:S_C8] = B[6]
    g4[TD_ONE, S_C8:S_C4] = B[8]
    g4[TD_ONE, S_C4:S_ONE] = B[4]
    g4[TD_ONE, S_ONE] = 1.0                       # ones passthrough

    return wt.astype(np.float16)


def _pack_core_inputs(a, b, meta, n_stream):
    """One core's shard -> (tA [128, ns*N], tB [128, ns*N]) fp16.

    Full 128-partition streams (pad rows zero); columns past the shard
    replicate column 0 so tail-step quad DMAs read valid data."""
    bc = a.shape[0]
    ns = n_stream * N
    tA = np.zeros((128, ns), np.float16)
    tA[TA_A:TA_ONE, :bc] = a.astype(np.float16).T
    tA[TA_ONE] = 1.0
    tB = np.zeros((128, ns), np.float16)
    tB[TB_B:TB_ONE, :bc] = b.astype(np.float16).T
    tB[TB_ONE] = 1.0
    m = meta.astype(np.float32)
    tB[TB_M:TB_END, :bc] = np.where(m > 0, m, m * 100.0).astype(np.float16).T
    tA[TA_A:TA_ONE, bc:] = tA[TA_A:TA_ONE, 0:1]
    tB[TB_B:TB_ONE, bc:] = tB[TB_B:TB_ONE, 0:1]
    tB[TB_M:TB_END, bc:] = tB[TB_M:TB_END, 0:1]
    return tA, tB


def build_bass(n_chunks):
    nc = bacc.Bacc(None, target_bir_lowering=False, debug=False)
    n_steps = n_chunks + PIPE                       # 74
    n_quads = (n_steps + 2 + QUAD - 1) // QUAD      # tiles 0..n_steps+1
    n_stream = n_quads * QUAD

    tA_d = nc.dram_tensor("tA", [128, n_stream * N], F16, kind="ExternalInput")
    tB_d = nc.dram_tensor("tB", [128, n_stream * N], F16, kind="ExternalInput")
    wt_d = nc.dram_tensor("wt", [128, 768], F16, kind="ExternalInput")
    y_d = nc.dram_tensor("y", [1, n_chunks * N], F16, kind="ExternalOutput")

    with tile.TileContext(nc) as tc:
        with (
            tc.tile_pool(name="const", bufs=1) as constp,
            tc.tile_pool(name="tap", bufs=3) as tap,
            tc.tile_pool(name="tbp", bufs=3) as tbp,
            tc.tile_pool(name="sp", bufs=4) as spool,
            tc.tile_pool(name="up", bufs=4) as up,
            tc.tile_pool(name="ps", bufs=2, space=bass.MemorySpace.PSUM) as ps,
        ):
            wt = constp.tile([128, 768], F16, tag="wt")
            nc.sync.dma_start(wt[:], wt_d[:])
            wg1i, wg1s, wg2 = wt[:, 0:128], wt[:, 128:256], wt[:, 256:384]
            wg5, wg3, wg4 = wt[:, 384:512], wt[:, 512:640], wt[:, 640:768]

            def chain(*insts):
                for i in range(1, len(insts)):
                    add_dep_helper(insts[i].ins, insts[i - 1].ins,
                                   sync=False, reason="psum acc order")

            TAq, TBq, S, U = {}, {}, {}, {}

            def new_quad(p):
                TAq[p] = tap.tile([128, QUAD * N], F16, tag="ta", name=f"ta_{p}")
                TBq[p] = tbp.tile([128, QUAD * N], F16, tag="tb", name=f"tb_{p}")
                qs = slice(p * QUAD * N, (p + 1) * QUAD * N)
                nc.sync.dma_start(TAq[p][0:128], tA_d[:, qs])
                nc.sync.dma_start(TBq[p][0:128], tB_d[:, qs])

            def new_state(t):
                S[t] = spool.tile([128, N], F16, tag="s", name=f"s_{t}")
                U[t] = up.tile([128, 2 * N], F16, tag="u", name=f"u_{t}")
                if t <= 1:
                    nc.gpsimd.memset(S[t][0:128], 0.0)
                    nc.gpsimd.memset(U[t][0:128], 0.0)

            new_quad(0)
            new_state(0)
            new_state(1)

            mm = nc.tensor.matmul
            for t in range(n_steps):
                if (t + 1) % QUAD == 0:
                    new_quad((t + 1) // QUAD)
                new_state(t + 2)
                ta = TAq[t // QUAD]
                tb = TBq[t // QUAD]
                cs = slice((t % QUAD) * N, (t % QUAD + 1) * N)

                qab = ps.tile([128, 2 * N], F32, tag="qab", name=f"qab_{t}")
                i1 = mm(qab[0:128, 0:N], wg1i[0:TA_END], ta[0:TA_END, cs],
                        start=True, stop=False, tile_position=(0, 0))
                i2 = mm(qab[0:128, 0:N], wg1s[0:S_END], S[t][0:S_END],
                        start=False, stop=True, tile_position=(0, 0))
                chain(i1, i2)
                i3 = mm(qab[0:128, N:2 * N], wg2[0:TB_END], tb[0:TB_END, cs],
                        start=True, stop=False, tile_position=(0, 0))
                i4 = mm(qab[0:128, N:2 * N], wg5[0:S_END], S[t][0:S_END],
                        start=False, stop=True, tile_position=(0, 0))
                chain(i3, i4)
                q34 = ps.tile([128, N], F32, tag="q34", name=f"q34_{t}")
                i5 = mm(q34[0:128], wg3[0:TC_END], U[t][0:TC_END, 0:N],
                        start=True, stop=False, tile_position=(0, 0))
                i6 = mm(q34[0:128], wg4[0:TD_END], U[t][0:TD_END, N:2 * N],
                        start=False, stop=True, tile_position=(0, 0))
                chain(i5, i6)

                # ---- drains into step t+1 tiles ----
                nc.scalar.activation(U[t + 2][0:128, 0:2 * N],
                                     qab[0:128, 0:2 * N], PRELU, alpha=ALPHA)
                nc.vector.tensor_copy(S[t + 2][0:S_END], q34[0:S_END])
                nc.vector.scalar_tensor_tensor(
                    S[t + 2][0:S_END], S[t + 2][0:S_END],
                    ALPHA, S[t + 2][0:S_END], ALU.mult, ALU.max)

                # ---- y out (chunk t-10 sits in U[t+1] row 127, TC half) ----
                if t >= PIPE:
                    c = t - PIPE
                    nc.gpsimd.dma_start(y_d[:, c * N:(c + 1) * N],
                                        U[t + 2][TC_Y:TC_END, 0:N])

    nc.compile()
    return nc


_NC_CACHE = {}


def _get_nc(n_chunks):
    if n_chunks not in _NC_CACHE:
        _NC_CACHE[n_chunks] = build_bass(n_chunks)
    return _NC_CACHE[n_chunks]


def run_cores(inputs, n_chunks, cores, trace=False, trace_kwargs=None):
    a = np.asarray(inputs["a"], np.float32)
    b = np.asarray(inputs["b"], np.float32)
    meta = np.asarray(inputs["meta"], np.float32)
    wt = _pack_weights(inputs)
    n_steps = n_chunks + PIPE
    n_stream = ((n_steps + 2 + QUAD - 1) // QUAD) * QUAD
    in_maps = []
    for r in cores:
        sl = slice(r * B_CORE, r * B_CORE + n_chunks * N)
        tA, tB = _pack_core_inputs(a[sl], b[sl], meta[sl], n_stream)
        in_maps.append({"tA": tA, "tB": tB, "wt": wt})
    nc = _get_nc(n_chunks)
    kw = dict(trace=trace)
    if trace_kwargs:
        kw.update(trace_kwargs)
    res = run_bass_kernel_spmd(nc, in_maps, list(range(len(cores))), **kw)
    return [res.results[i]["y"] for i in range(len(cores))], res


def kernel(**inputs):
    n_chunks = B_CORE // N
    ys, _ = run_cores(inputs, n_chunks, list(range(N_CORES)))
    out = np.empty((B_FULL, 1), np.float32)
    for r in range(N_CORES):
        out[r * B_CORE:(r + 1) * B_CORE, 0] = ys[r][0].astype(np.float32)
    return out


# revision 13
# speedup vs baseline: 1.2925x; 1.2925x over previous
"""Trainium2 Bass kernel for nn_Net_67954972557347 (dense_mlp).

Network: a1 = lrelu(a@Wa+ba) [B,68]; b1 = lrelu(b@Wb+bb) [B,68];
c = [a1|b1|meta] [B,140]; then 10 lrelu'd dense layers
(140->34->34->20->20->20->20->20->5->2->1), lrelu slope 0.01.

Strategy: pure data parallel over 8 cores (32768 rows each), activations
feature-major ([feat, batch]), batch streamed 512 columns per chunk.
All tensors fp16 (same 10-bit mantissa as tf32; fp32 PSUM accumulation).
This box's power manager clamps the PE to 1.2 GHz under sustained load,
so matmul-pass count dominates: the 12 layers are packed into FOUR
matmuls per chunk via a software pipeline (one layer-hop per 2 steps;
stages of different chunks colocate in one moving tile / stationary):

  TA [c0(34); c2(20); c6(20); c8(2); a(45); ones] (quad tile)
    --G1--> Qab bank0: [a1(68); c1(34); c3(20); c7(5); y(1)] = 128
  TB [c4(20); b(102); ones; ilrelu(meta)(4)]      (quad tile)
    --G2--> Qab bank1: [b1(68); c5(20); meta(4); one(1)] (passthroughs)
  U[:, 0:512] = TC = bank0 drained;  U[:, 512:1024] = TD = bank1 drained
  U-TC [128p] --G3--> Q34 [c0p; c2; -; c8; pad; c4@96] (start)
  U-TD [93p]  --G4--> Q34 [c0p; -; c6; -; -]           (accum, stop)

Input DMAs are full 128-partition 4-chunk quads (partial-partition DMAs
run ~10x slower); the DMA pre-fills the drain-owned c-rows with zeros
and the per-step drains overwrite them afterwards. Drains lag TWO steps
(write tiles consumed at t+2), which splits the pipeline into two
independent interleaved chains so every matmul's inputs are ready at
step start (no inter-engine ping-pong on the critical path):

  Qab[0:128, 0:1024] -> U[t+2]          one ScalarE Prelu (2 banks)
  Q34[0:76]          -> TA'[0:76]       ScalarE Prelu
  Q34[96:116]        -> TB'[0:20]       VectorE cast + stt lrelu

All biases ride ones rows; meta and the TD-one ride G2 as identity
passthroughs (host pre-applies inverse-lrelu). y is DMA'd from U row
127 on the gpsimd queue.
"""

import os
import sys

import numpy as np

for _p in ("/opt/trn_rl_repo", "/root/.axon_site/_ro/trn_rl_repo"):
    if os.path.isdir(_p) and _p not in sys.path:
        sys.path.append(_p)

import concourse.bass as bass
import concourse.mybir as mybir
import concourse.tile as tile
from concourse import bacc
from concourse.bass_utils import run_bass_kernel_spmd
from bass_rust import add_dep_helper

F16 = mybir.dt.float16
F32 = mybir.dt.float32
ALU = mybir.AluOpType
PRELU = mybir.ActivationFunctionType.Prelu

B_FULL = 262144
N_CORES = 8
B_CORE = B_FULL // N_CORES          # 32768
N = 512                              # columns per chunk (PSUM fp32 cap)
PIPE = 20                            # pipeline depth (2 steps per layer hop)
ALPHA = 0.01
QUAD = 4                             # chunks per input tile / DMA

# TA rows
TA_C0, TA_C2, TA_C6, TA_C8, TA_CE, TA_A, TA_ONE, TA_END = \
    0, 34, 54, 74, 76, 76, 121, 122
# TB rows
TB_C4, TB_B, TB_ONE, TB_M, TB_END = 0, 20, 122, 123, 127
# U cols 0:512 = TC rows (== Qab bank0 cols)
TC_A1, TC_C1, TC_C3, TC_C7, TC_Y, TC_END = 0, 68, 102, 122, 127, 128
# U cols 512:1024 = TD rows (== Qab bank1 cols)
TD_B1, TD_C5, TD_M, TD_ONE, TD_END = 0, 68, 88, 92, 93
# Q34 column layout (pad 76:96 so the c4 window starts at partition 96)
Q34_C0, Q34_C2, Q34_C6, Q34_C8, Q34_CE, Q34_C4, Q34_C4E = \
    0, 34, 54, 74, 76, 96, 116


def _pack_weights(inp):
    """Four [128,128] fp16 stationaries packed into one [128, 512] tile."""
    f32 = lambda k: np.asarray(inp[k], np.float32)
    Wa, ba, Wb, bb = f32("Wa"), f32("ba"), f32("Wb"), f32("bb")
    W = [f32(f"W{i}") for i in range(10)]
    B = [f32(f"B{i}") for i in range(10)]
    wt = np.zeros((128, 512), np.float32)

    g1 = wt[:, 0:128]
    g1[TA_C0:TA_C2, TC_C1:TC_C3] = W[1]
    g1[TA_C2:TA_C6, TC_C3:TC_C7] = W[3]
    g1[TA_C6:TA_C8, TC_C7:TC_Y] = W[7]
    g1[TA_C8:TA_CE, TC_Y:TC_END] = W[9]
    g1[TA_A:TA_ONE, TC_A1:TC_C1] = Wa
    g1[TA_ONE, TC_A1:TC_C1] = ba
    g1[TA_ONE, TC_C1:TC_C3] = B[1]
    g1[TA_ONE, TC_C3:TC_C7] = B[3]
    g1[TA_ONE, TC_C7:TC_Y] = B[7]
    g1[TA_ONE, TC_Y:TC_END] = B[9]

    g2 = wt[:, 128:256]
    g2[TB_C4:TB_B, TD_C5:TD_M] = W[5]
    g2[TB_B:TB_ONE, TD_B1:TD_C5] = Wb
    g2[TB_ONE, TD_B1:TD_C5] = bb
    g2[TB_ONE, TD_C5:TD_M] = B[5]
    g2[TB_ONE, TD_ONE] = 1.0                      # ones passthrough
    g2[TB_M:TB_END, TD_M:TD_ONE] = np.eye(4)      # meta passthrough

    g3 = wt[:, 256:384]
    g3[TC_A1:TC_C1, Q34_C0:Q34_C2] = W[0][0:68]
    g3[TC_C1:TC_C3, Q34_C2:Q34_C6] = W[2]
    g3[TC_C3:TC_C7, Q34_C4:Q34_C4E] = W[4]
    g3[TC_C7:TC_Y, Q34_C8:Q34_CE] = W[8]

    g4 = wt[:, 384:512]
    g4[TD_B1:TD_C5, Q34_C0:Q34_C2] = W[0][68:136]
    g4[TD_C5:TD_M, Q34_C6:Q34_C8] = W[6]
    g4[TD_M:TD_ONE, Q34_C0:Q34_C2] = W[0][136:140]
    g4[TD_ONE, Q34_C0:Q34_C2] = B[0]
    g4[TD_ONE, Q34_C2:Q34_C6] = B[2]
    g4[TD_ONE, Q34_C6:Q34_C8] = B[6]
    g4[TD_ONE, Q34_C8:Q34_CE] = B[8]
    g4[TD_ONE, Q34_C4:Q34_C4E] = B[4]

    return wt.astype(np.float16)


def _pack_core_inputs(a, b, meta, n_stream):
    """One core's shard -> (tA [128, ns*N], tB [128, ns*N]) fp16.

    Full 128-partition streams. Drain-owned rows (TA[0:76], TB[0:20])
    ship as zeros and are overwritten on-device; columns past the shard
    replicate column 0 so tail-step quad DMAs read valid data."""
    bc = a.shape[0]
    ns = n_stream * N
    tA = np.zeros((128, ns), np.float16)
    tA[TA_A:TA_ONE, :bc] = a.astype(np.float16).T
    tA[TA_ONE] = 1.0
    tB = np.zeros((128, ns), np.float16)
    tB[TB_B:TB_ONE, :bc] = b.astype(np.float16).T
    tB[TB_ONE] = 1.0
    m = meta.astype(np.float32)
    tB[TB_M:TB_END, :bc] = np.where(m > 0, m, m * 100.0).astype(np.float16).T
    tA[TA_A:TA_ONE, bc:] = tA[TA_A:TA_ONE, 0:1]
    tB[TB_B:TB_ONE, bc:] = tB[TB_B:TB_ONE, 0:1]
    tB[TB_M:TB_END, bc:] = tB[TB_M:TB_END, 0:1]
    return tA, tB


def build_bass(n_chunks):
    nc = bacc.Bacc(None, target_bir_lowering=False, debug=False)
    n_steps = n_chunks + PIPE                       # 84
    n_quads = (n_steps + 2 + QUAD - 1) // QUAD      # tiles 0..n_steps+1
    n_stream = n_quads * QUAD

    tA_d = nc.dram_tensor("tA", [128, n_stream * N], F16, kind="ExternalInput")
    tB_d = nc.dram_tensor("tB", [128, n_stream * N], F16, kind="ExternalInput")
    wt_d = nc.dram_tensor("wt", [128, 512], F16, kind="ExternalInput")
    y_d = nc.dram_tensor("y", [1, n_chunks * N], F16, kind="ExternalOutput")

    with tile.TileContext(nc) as tc:
        with (
            tc.tile_pool(name="const", bufs=1) as constp,
            tc.tile_pool(name="tap", bufs=3) as tap,
            tc.tile_pool(name="tbp", bufs=3) as tbp,
            tc.tile_pool(name="up", bufs=4) as up,
            tc.tile_pool(name="ps", bufs=2, space=bass.MemorySpace.PSUM) as ps,
        ):
            wt = constp.tile([128, 512], F16, tag="wt")
            nc.sync.dma_start(wt[:], wt_d[:])
            wg1, wg2 = wt[:, 0:128], wt[:, 128:256]
            wg3, wg4 = wt[:, 256:384], wt[:, 384:512]

            def chain(*insts):
                for i in range(1, len(insts)):
                    add_dep_helper(insts[i].ins, insts[i - 1].ins,
                                   sync=False, reason="psum acc order")

            TAq, TBq, U = {}, {}, {}

            def new_quad(p):
                if p >= n_quads:
                    return
                TAq[p] = tap.tile([128, QUAD * N], F16, tag="ta", name=f"ta_{p}")
                TBq[p] = tbp.tile([128, QUAD * N], F16, tag="tb", name=f"tb_{p}")
                qs = slice(p * QUAD * N, (p + 1) * QUAD * N)
                nc.sync.dma_start(TAq[p][0:128], tA_d[:, qs])
                nc.sync.dma_start(TBq[p][0:128], tB_d[:, qs])

            def new_u(t):
                U[t] = up.tile([128, 2 * N], F16, tag="u", name=f"u_{t}")
                if t <= 1:
                    nc.gpsimd.memset(U[t][0:128], 0.0)

            new_quad(0)
            new_quad(1)
            new_u(0)
            new_u(1)

            mm = nc.tensor.matmul
            for t in range(n_steps):
                if (t + 6) % QUAD == 0:
                    new_quad((t + 6) // QUAD)
                new_u(t + 2)
                ta = TAq[t // QUAD]
                tb = TBq[t // QUAD]
                cs = slice((t % QUAD) * N, (t % QUAD + 1) * N)
                dt = t + 2                           # drain target step
                dta = TAq[dt // QUAD]
                dtb = TBq[dt // QUAD]
                dcs = slice((dt % QUAD) * N, (dt % QUAD + 1) * N)

                qab = ps.tile([128, 2 * N], F32, tag="qab", name=f"qab_{t}")
                mm(qab[0:128, 0:N], wg1[0:TA_END], ta[0:TA_END, cs],
                   start=True, stop=True, tile_position=(0, 0))
                mm(qab[0:128, N:2 * N], wg2[0:TB_END], tb[0:TB_END, cs],
                   start=True, stop=True, tile_position=(0, 0))
                q34 = ps.tile([128, N], F32, tag="q34", name=f"q34_{t}")
                i3 = mm(q34[0:128], wg3[0:TC_END], U[t][0:TC_END, 0:N],
                        start=True, stop=False, tile_position=(0, 0))
                i4 = mm(q34[0:128], wg4[0:TD_END], U[t][0:TD_END, N:2 * N],
                        start=False, stop=True, tile_position=(0, 0))
                chain(i3, i4)

                # ---- drains into step t+2 tiles ----
                nc.scalar.activation(U[t + 2][0:128, 0:2 * N],
                                     qab[0:128, 0:2 * N], PRELU, alpha=ALPHA)
                nc.scalar.activation(dta[TA_C0:TA_CE, dcs],
                                     q34[Q34_C0:Q34_CE], PRELU, alpha=ALPHA)
                nc.vector.tensor_copy(dtb[TB_C4:TB_B, dcs],
                                      q34[Q34_C4:Q34_C4E])
                nc.vector.scalar_tensor_tensor(
                    dtb[TB_C4:TB_B, dcs], dtb[TB_C4:TB_B, dcs],
                    ALPHA, dtb[TB_C4:TB_B, dcs], ALU.mult, ALU.max)

                # ---- y out (chunk t-20 sits in U[t+2] row 127, TC half) ----
                if t >= PIPE:
                    c = t - PIPE
                    nc.gpsimd.dma_start(y_d[:, c * N:(c + 1) * N],
                                        U[t + 2][TC_Y:TC_END, 0:N])

    nc.compile()
    return nc


_NC_CACHE = {}


def _get_nc(n_chunks):
    if n_chunks not in _NC_CACHE:
        _NC_CACHE[n_chunks] = build_bass(n_chunks)
    return _NC_CACHE[n_chunks]


def run_cores(inputs, n_chunks, cores, trace=False, trace_kwargs=None):
    a = np.asarray(inputs["a"], np.float32)
    b = np.asarray(inputs["b"], np.float32)
    meta = np.asarray(inputs["meta"], np.float32)
    wt = _pack_weights(inputs)
    n_steps = n_chunks + PIPE
    n_stream = ((n_steps + 2 + QUAD - 1) // QUAD) * QUAD
    in_maps = []
    for r in cores:
        sl = slice(r * B_CORE, r * B_CORE + n_chunks * N)
        tA, tB = _pack_core_inputs(a[sl], b[sl], meta[sl], n_stream)
        in_maps.append({"tA": tA, "tB": tB, "wt": wt})
    nc = _get_nc(n_chunks)
    kw = dict(trace=trace)
    if trace_kwargs:
        kw.update(trace_kwargs)
    res = run_bass_kernel_spmd(nc, in_maps, list(range(len(cores))), **kw)
    return [res.results[i]["y"] for i in range(len(cores))], res


def kernel(**inputs):
    n_chunks = B_CORE // N
    ys, _ = run_cores(inputs, n_chunks, list(range(N_CORES)))
    out = np.empty((B_FULL, 1), np.float32)
    for r in range(N_CORES):
        out[r * B_CORE:(r + 1) * B_CORE, 0] = ys[r][0].astype(np.float32)
    return out


# revision 14
# speedup vs baseline: 1.5381x; 1.1900x over previous
"""Trainium2 Bass kernel for nn_Net_67954972557347 (dense_mlp).

Network: a1 = lrelu(a@Wa+ba) [B,68]; b1 = lrelu(b@Wb+bb) [B,68];
c = [a1|b1|meta] [B,140]; then 10 lrelu'd dense layers
(140->34->34->20->20->20->20->20->5->2->1), lrelu slope 0.01.

Strategy: pure data parallel over 8 cores (32768 rows each), activations
feature-major ([feat, batch]), batch streamed 512 columns per chunk.
All tensors fp16 (same 10-bit mantissa as tf32; fp32 PSUM accumulation).
This box's power manager clamps the PE to 1.2 GHz under sustained load,
so matmul-pass count dominates: the 12 layers are packed into FOUR
matmuls per chunk via a software pipeline (one layer-hop per 2 steps;
stages of different chunks colocate in one moving tile / stationary):

  TA [c0(34); c2(20); c6(20); c8(2); a(45); ones] (quad tile)
    --G1--> Qab bank0: [a1(68); c1(34); c3(20); c7(5); y(1)] = 128
  TB [c4(20); b(102); ones; ilrelu(meta)(4)]      (quad tile)
    --G2--> Qab bank1: [b1(68); c5(20); meta(4); one(1)] (passthroughs)
  U[:, 0:512] = TC = bank0 drained;  U[:, 512:1024] = TD = bank1 drained
  U-TC [128p] --G3--> Q34 [c0p; c2; -; c8; pad; c4@96] (start)
  U-TD [93p]  --G4--> Q34 [c0p; -; c6; -; -]           (accum, stop)

Input DMAs are full 128-partition 4-chunk quads (partial-partition DMAs
run ~10x slower); the DMA pre-fills the drain-owned c-rows with zeros
and the per-step drains overwrite them afterwards. Drains lag TWO steps
(write tiles consumed at t+2), which splits the pipeline into two
independent interleaved chains so every matmul's inputs are ready at
step start (no inter-engine ping-pong on the critical path):

  Qab[0:128, 0:1024] -> U[t+2]          one ScalarE Prelu (2 banks)
  Q34[0:76]          -> TA'[0:76]       ScalarE Prelu
  Q34[96:116]        -> TB'[0:20]       VectorE cast + stt lrelu

All biases ride ones rows; meta and the TD-one ride G2 as identity
passthroughs (host pre-applies inverse-lrelu). y is DMA'd from U row
127 on the gpsimd queue.
"""

import os
import sys

import numpy as np

for _p in ("/opt/trn_rl_repo", "/root/.axon_site/_ro/trn_rl_repo"):
    if os.path.isdir(_p) and _p not in sys.path:
        sys.path.append(_p)

import concourse.bass as bass
import concourse.mybir as mybir
import concourse.tile as tile
from concourse import bacc
from concourse.bass_utils import run_bass_kernel_spmd
from bass_rust import add_dep_helper

F16 = mybir.dt.float16
F32 = mybir.dt.float32
ALU = mybir.AluOpType
PRELU = mybir.ActivationFunctionType.Prelu

B_FULL = 262144
N_CORES = 8
B_CORE = B_FULL // N_CORES          # 32768
N = 512                              # columns per chunk (PSUM fp32 cap)
PIPE = 20                            # pipeline depth (2 steps per layer hop)
ALPHA = 0.01
QUAD = 4                             # chunks per input tile / DMA

# TA rows
TA_C0, TA_C2, TA_C6, TA_C8, TA_CE, TA_A, TA_ONE, TA_END = \
    0, 34, 54, 74, 76, 76, 121, 122
# TB rows
TB_C4, TB_B, TB_ONE, TB_M, TB_END = 0, 20, 122, 123, 127
# U cols 0:512 = TC rows (== Qab bank0 cols)
TC_A1, TC_C1, TC_C3, TC_C7, TC_Y, TC_END = 0, 68, 102, 122, 127, 128
# U cols 512:1024 = TD rows (== Qab bank1 cols)
TD_B1, TD_C5, TD_M, TD_ONE, TD_END = 0, 68, 88, 92, 93
# Q34 column layout (pad 76:96 so the c4 window starts at partition 96)
Q34_C0, Q34_C2, Q34_C6, Q34_C8, Q34_CE, Q34_C4, Q34_C4E = \
    0, 34, 54, 74, 76, 96, 116


def _pack_weights(inp):
    """Four [128,128] fp16 stationaries packed into one [128, 512] tile."""
    f32 = lambda k: np.asarray(inp[k], np.float32)
    Wa, ba, Wb, bb = f32("Wa"), f32("ba"), f32("Wb"), f32("bb")
    W = [f32(f"W{i}") for i in range(10)]
    B = [f32(f"B{i}") for i in range(10)]
    wt = np.zeros((128, 512), np.float32)

    g1 = wt[:, 0:128]
    g1[TA_C0:TA_C2, TC_C1:TC_C3] = W[1]
    g1[TA_C2:TA_C6, TC_C3:TC_C7] = W[3]
    g1[TA_C6:TA_C8, TC_C7:TC_Y] = W[7]
    g1[TA_C8:TA_CE, TC_Y:TC_END] = W[9]
    g1[TA_A:TA_ONE, TC_A1:TC_C1] = Wa
    g1[TA_ONE, TC_A1:TC_C1] = ba
    g1[TA_ONE, TC_C1:TC_C3] = B[1]
    g1[TA_ONE, TC_C3:TC_C7] = B[3]
    g1[TA_ONE, TC_C7:TC_Y] = B[7]
    g1[TA_ONE, TC_Y:TC_END] = B[9]

    g2 = wt[:, 128:256]
    g2[TB_C4:TB_B, TD_C5:TD_M] = W[5]
    g2[TB_B:TB_ONE, TD_B1:TD_C5] = Wb
    g2[TB_ONE, TD_B1:TD_C5] = bb
    g2[TB_ONE, TD_C5:TD_M] = B[5]
    g2[TB_ONE, TD_ONE] = 1.0                      # ones passthrough
    g2[TB_M:TB_END, TD_M:TD_ONE] = np.eye(4)      # meta passthrough

    g3 = wt[:, 256:384]
    g3[TC_A1:TC_C1, Q34_C0:Q34_C2] = W[0][0:68]
    g3[TC_C1:TC_C3, Q34_C2:Q34_C6] = W[2]
    g3[TC_C3:TC_C7, Q34_C4:Q34_C4E] = W[4]
    g3[TC_C7:TC_Y, Q34_C8:Q34_CE] = W[8]

    g4 = wt[:, 384:512]
    g4[TD_B1:TD_C5, Q34_C0:Q34_C2] = W[0][68:136]
    g4[TD_C5:TD_M, Q34_C6:Q34_C8] = W[6]
    g4[TD_M:TD_ONE, Q34_C0:Q34_C2] = W[0][136:140]
    g4[TD_ONE, Q34_C0:Q34_C2] = B[0]
    g4[TD_ONE, Q34_C2:Q34_C6] = B[2]
    g4[TD_ONE, Q34_C6:Q34_C8] = B[6]
    g4[TD_ONE, Q34_C8:Q34_CE] = B[8]
    g4[TD_ONE, Q34_C4:Q34_C4E] = B[4]

    return wt.astype(np.float16)


def _pack_core_inputs(a, b, meta, n_stream):
    """One core's shard -> (tA [128, ns*N], tB [128, ns*N]) fp16.

    Full 128-partition streams. Drain-owned rows (TA[0:76], TB[0:20])
    ship as zeros and are overwritten on-device; columns past the shard
    replicate column 0 so tail-step quad DMAs read valid data."""
    bc = a.shape[0]
    ns = n_stream * N
    tA = np.zeros((128, ns), np.float16)
    tA[TA_A:TA_ONE, :bc] = a.astype(np.float16).T
    tA[TA_ONE] = 1.0
    tB = np.zeros((128, ns), np.float16)
    tB[TB_B:TB_ONE, :bc] = b.astype(np.float16).T
    tB[TB_ONE] = 1.0
    m = meta.astype(np.float32)
    tB[TB_M:TB_END, :bc] = np.where(m > 0, m, m * 100.0).astype(np.float16).T
    tA[TA_A:TA_ONE, bc:] = tA[TA_A:TA_ONE, 0:1]
    tB[TB_B:TB_ONE, bc:] = tB[TB_B:TB_ONE, 0:1]
    tB[TB_M:TB_END, bc:] = tB[TB_M:TB_END, 0:1]
    return tA, tB


def build_bass(n_chunks):
    nc = bacc.Bacc(None, target_bir_lowering=False, debug=False)
    n_steps = n_chunks + PIPE                       # 84
    n_quads = (n_steps + 2 + QUAD - 1) // QUAD      # tiles 0..n_steps+1
    n_stream = n_quads * QUAD

    tA_d = nc.dram_tensor("tA", [128, n_stream * N], F16, kind="ExternalInput")
    tB_d = nc.dram_tensor("tB", [128, n_stream * N], F16, kind="ExternalInput")
    wt_d = nc.dram_tensor("wt", [128, 512], F16, kind="ExternalInput")
    y_d = nc.dram_tensor("y", [1, n_chunks * N], F16, kind="ExternalOutput")

    with tile.TileContext(nc) as tc:
        with (
            tc.tile_pool(name="const", bufs=1) as constp,
            tc.tile_pool(name="tap", bufs=4) as tap,
            tc.tile_pool(name="tbp", bufs=4) as tbp,
            tc.tile_pool(name="up", bufs=6) as up,
            tc.tile_pool(name="ps", bufs=2, space=bass.MemorySpace.PSUM) as ps,
        ):
            wt = constp.tile([128, 512], F16, tag="wt")
            nc.sync.dma_start(wt[:], wt_d[:])
            wg1, wg2 = wt[:, 0:128], wt[:, 128:256]
            wg3, wg4 = wt[:, 256:384], wt[:, 384:512]

            def chain(*insts):
                for i in range(1, len(insts)):
                    add_dep_helper(insts[i].ins, insts[i - 1].ins,
                                   sync=False, reason="psum acc order")

            TAq, TBq, U = {}, {}, {}

            def new_quad(p):
                if p >= n_quads:
                    return
                TAq[p] = tap.tile([128, QUAD * N], F16, tag="ta", name=f"ta_{p}")
                TBq[p] = tbp.tile([128, QUAD * N], F16, tag="tb", name=f"tb_{p}")
                qs = slice(p * QUAD * N, (p + 1) * QUAD * N)
                nc.sync.dma_start(TAq[p][0:128], tA_d[:, qs])
                nc.sync.dma_start(TBq[p][0:128], tB_d[:, qs])

            def new_u(t):
                U[t] = up.tile([128, 2 * N], F16, tag="u", name=f"u_{t}")
                if t <= 1:
                    nc.gpsimd.memset(U[t][0:128], 0.0)

            new_quad(0)
            new_quad(1)
            new_u(0)
            new_u(1)

            mm = nc.tensor.matmul
            for t in range(n_steps):
                if (t + 6) % QUAD == 0:
                    new_quad((t + 6) // QUAD)
                new_u(t + 2)
                ta = TAq[t // QUAD]
                tb = TBq[t // QUAD]
                cs = slice((t % QUAD) * N, (t % QUAD + 1) * N)
                dt = t + 2                           # drain target step
                dta = TAq[dt // QUAD]
                dtb = TBq[dt // QUAD]
                dcs = slice((dt % QUAD) * N, (dt % QUAD + 1) * N)

                qab = ps.tile([128, 2 * N], F32, tag="qab", name=f"qab_{t}")
                mm(qab[0:128, 0:N], wg1[0:TA_END], ta[0:TA_END, cs],
                   start=True, stop=True, tile_position=(0, 0))
                mm(qab[0:128, N:2 * N], wg2[0:TB_END], tb[0:TB_END, cs],
                   start=True, stop=True, tile_position=(0, 0))
                q34 = ps.tile([128, N], F32, tag="q34", name=f"q34_{t}")
                i3 = mm(q34[0:128], wg3[0:TC_END], U[t][0:TC_END, 0:N],
                        start=True, stop=False, tile_position=(0, 0))
                i4 = mm(q34[0:128], wg4[0:TD_END], U[t][0:TD_END, N:2 * N],
                        start=False, stop=True, tile_position=(0, 0))
                chain(i3, i4)

                # ---- drains into step t+2 tiles ----
                nc.scalar.activation(U[t + 2][0:128, 0:2 * N],
                                     qab[0:128, 0:2 * N], PRELU, alpha=ALPHA)
                nc.scalar.activation(dta[TA_C0:TA_CE, dcs],
                                     q34[Q34_C0:Q34_CE], PRELU, alpha=ALPHA)
                nc.vector.tensor_copy(dtb[TB_C4:TB_B, dcs],
                                      q34[Q34_C4:Q34_C4E])
                nc.vector.scalar_tensor_tensor(
                    dtb[TB_C4:TB_B, dcs], dtb[TB_C4:TB_B, dcs],
                    ALPHA, dtb[TB_C4:TB_B, dcs], ALU.mult, ALU.max)

                # ---- y out (chunk t-20 sits in U[t+2] row 127, TC half) ----
                if t >= PIPE:
                    c = t - PIPE
                    nc.gpsimd.dma_start(y_d[:, c * N:(c + 1) * N],
                                        U[t + 2][TC_Y:TC_END, 0:N])

    nc.compile()
    return nc


_NC_CACHE = {}


def _get_nc(n_chunks):
    if n_chunks not in _NC_CACHE:
        _NC_CACHE[n_chunks] = build_bass(n_chunks)
    return _NC_CACHE[n_chunks]


def run_cores(inputs, n_chunks, cores, trace=False, trace_kwargs=None):
    a = np.asarray(inputs["a"], np.float32)
    b = np.asarray(inputs["b"], np.float32)
    meta = np.asarray(inputs["meta"], np.float32)
    wt = _pack_weights(inputs)
    n_steps = n_chunks + PIPE
    n_stream = ((n_steps + 2 + QUAD - 1) // QUAD) * QUAD
    in_maps = []
    for r in cores:
        sl = slice(r * B_CORE, r * B_CORE + n_chunks * N)
        tA, tB = _pack_core_inputs(a[sl], b[sl], meta[sl], n_stream)
        in_maps.append({"tA": tA, "tB": tB, "wt": wt})
    nc = _get_nc(n_chunks)
    kw = dict(trace=trace)
    if trace_kwargs:
        kw.update(trace_kwargs)
    res = run_bass_kernel_spmd(nc, in_maps, list(range(len(cores))), **kw)
    return [res.results[i]["y"] for i in range(len(cores))], res


def kernel(**inputs):
    n_chunks = B_CORE // N
    ys, _ = run_cores(inputs, n_chunks, list(range(N_CORES)))
    out = np.empty((B_FULL, 1), np.float32)
    for r in range(N_CORES):
        out[r * B_CORE:(r + 1) * B_CORE, 0] = ys[r][0].astype(np.float32)
    return out


# revision 15
# speedup vs baseline: 1.5442x; 1.0040x over previous
"""Trainium2 Bass kernel for nn_Net_67954972557347 (dense_mlp).

Network: a1 = lrelu(a@Wa+ba) [B,68]; b1 = lrelu(b@Wb+bb) [B,68];
c = [a1|b1|meta] [B,140]; then 10 lrelu'd dense layers
(140->34->34->20->20->20->20->20->5->2->1), lrelu slope 0.01.

Strategy: pure data parallel over 8 cores (32768 rows each), activations
feature-major ([feat, batch]), batch streamed 512 columns per chunk.
All tensors fp16 (same 10-bit mantissa as tf32; fp32 PSUM accumulation).
This box's power manager clamps the PE to 1.2 GHz under sustained load,
so matmul-pass count dominates: the 12 layers are packed into FOUR
matmuls per chunk via a software pipeline (one layer-hop per 2 steps;
stages of different chunks colocate in one moving tile / stationary):

  TA [c0(34); c2(20); c6(20); c8(2); a(45); ones] (quad tile)
    --G1--> Qab bank0: [a1(68); c1(34); c3(20); c7(5); y(1)] = 128
  TB [c4(20); b(102); ones; ilrelu(meta)(4)]      (quad tile)
    --G2--> Qab bank1: [b1(68); c5(20); meta(4); one(1)] (passthroughs)
  U[:, 0:512] = TC = bank0 drained;  U[:, 512:1024] = TD = bank1 drained
  U-TC [128p] --G3--> Q34 [c0p; c2; -; c8; pad; c4@96] (start)
  U-TD [93p]  --G4--> Q34 [c0p; -; c6; -; -]           (accum, stop)

Input DMAs are full 128-partition 4-chunk quads (partial-partition DMAs
run ~10x slower); the DMA pre-fills the drain-owned c-rows with zeros
and the per-step drains overwrite them afterwards. Drains lag TWO steps
(write tiles consumed at t+2), which splits the pipeline into two
independent interleaved chains so every matmul's inputs are ready at
step start (no inter-engine ping-pong on the critical path):

  Qab[0:128, 0:1024] -> U[t+2]          one ScalarE Prelu (2 banks)
  Q34[0:76]          -> TA'[0:76]       ScalarE Prelu
  Q34[96:116]        -> TB'[0:20]       VectorE cast + stt lrelu

All biases ride ones rows; meta and the TD-one ride G2 as identity
passthroughs (host pre-applies inverse-lrelu). y is DMA'd from U row
127 on the gpsimd queue.
"""

import os
import sys

import numpy as np

for _p in ("/opt/trn_rl_repo", "/root/.axon_site/_ro/trn_rl_repo"):
    if os.path.isdir(_p) and _p not in sys.path:
        sys.path.append(_p)

import concourse.bass as bass
import concourse.mybir as mybir
import concourse.tile as tile
from concourse import bacc
from concourse.bass_utils import run_bass_kernel_spmd
from bass_rust import add_dep_helper

F16 = mybir.dt.float16
F32 = mybir.dt.float32
ALU = mybir.AluOpType
PRELU = mybir.ActivationFunctionType.Prelu

B_FULL = 262144
N_CORES = 8
B_CORE = B_FULL // N_CORES          # 32768
N = 512                              # columns per chunk (PSUM fp32 cap)
PIPE = 20                            # pipeline depth (2 steps per layer hop)
ALPHA = 0.01
QUAD = 4                             # chunks per input tile / DMA

# TA rows
TA_C0, TA_C2, TA_C6, TA_C8, TA_CE, TA_A, TA_ONE, TA_END = \
    0, 34, 54, 74, 76, 76, 121, 122
# TB rows
TB_C4, TB_B, TB_ONE, TB_M, TB_END = 0, 20, 122, 123, 127
# U cols 0:512 = TC rows (== Qab bank0 cols)
TC_A1, TC_C1, TC_C3, TC_C7, TC_Y, TC_END = 0, 68, 102, 122, 127, 128
# U cols 512:1024 = TD rows (== Qab bank1 cols)
TD_B1, TD_C5, TD_M, TD_ONE, TD_END = 0, 68, 88, 92, 93
# Q34 column layout (pad 76:96 so the c4 window starts at partition 96)
Q34_C0, Q34_C2, Q34_C6, Q34_C8, Q34_CE, Q34_C4, Q34_C4E = \
    0, 34, 54, 74, 76, 96, 116


def _pack_weights(inp):
    """Four [128,128] fp16 stationaries packed into one [128, 512] tile."""
    f32 = lambda k: np.asarray(inp[k], np.float32)
    Wa, ba, Wb, bb = f32("Wa"), f32("ba"), f32("Wb"), f32("bb")
    W = [f32(f"W{i}") for i in range(10)]
    B = [f32(f"B{i}") for i in range(10)]
    wt = np.zeros((128, 512), np.float32)

    g1 = wt[:, 0:128]
    g1[TA_C0:TA_C2, TC_C1:TC_C3] = W[1]
    g1[TA_C2:TA_C6, TC_C3:TC_C7] = W[3]
    g1[TA_C6:TA_C8, TC_C7:TC_Y] = W[7]
    g1[TA_C8:TA_CE, TC_Y:TC_END] = W[9]
    g1[TA_A:TA_ONE, TC_A1:TC_C1] = Wa
    g1[TA_ONE, TC_A1:TC_C1] = ba
    g1[TA_ONE, TC_C1:TC_C3] = B[1]
    g1[TA_ONE, TC_C3:TC_C7] = B[3]
    g1[TA_ONE, TC_C7:TC_Y] = B[7]
    g1[TA_ONE, TC_Y:TC_END] = B[9]

    g2 = wt[:, 128:256]
    g2[TB_C4:TB_B, TD_C5:TD_M] = W[5]
    g2[TB_B:TB_ONE, TD_B1:TD_C5] = Wb
    g2[TB_ONE, TD_B1:TD_C5] = bb
    g2[TB_ONE, TD_C5:TD_M] = B[5]
    g2[TB_ONE, TD_ONE] = 1.0                      # ones passthrough
    g2[TB_M:TB_END, TD_M:TD_ONE] = np.eye(4)      # meta passthrough

    g3 = wt[:, 256:384]
    g3[TC_A1:TC_C1, Q34_C0:Q34_C2] = W[0][0:68]
    g3[TC_C1:TC_C3, Q34_C2:Q34_C6] = W[2]
    g3[TC_C3:TC_C7, Q34_C4:Q34_C4E] = W[4]
    g3[TC_C7:TC_Y, Q34_C8:Q34_CE] = W[8]

    g4 = wt[:, 384:512]
    g4[TD_B1:TD_C5, Q34_C0:Q34_C2] = W[0][68:136]
    g4[TD_C5:TD_M, Q34_C6:Q34_C8] = W[6]
    g4[TD_M:TD_ONE, Q34_C0:Q34_C2] = W[0][136:140]
    g4[TD_ONE, Q34_C0:Q34_C2] = B[0]
    g4[TD_ONE, Q34_C2:Q34_C6] = B[2]
    g4[TD_ONE, Q34_C6:Q34_C8] = B[6]
    g4[TD_ONE, Q34_C8:Q34_CE] = B[8]
    g4[TD_ONE, Q34_C4:Q34_C4E] = B[4]

    return wt.astype(np.float16)


def _pack_core_inputs(a, b, meta, n_stream):
    """One core's shard -> (tA [128, ns*N], tB [128, ns*N]) fp16.

    Full 128-partition streams. Drain-owned rows (TA[0:76], TB[0:20])
    ship as zeros and are overwritten on-device; columns past the shard
    replicate column 0 so tail-step quad DMAs read valid data."""
    bc = a.shape[0]
    ns = n_stream * N
    tA = np.zeros((128, ns), np.float16)
    tA[TA_A:TA_ONE, :bc] = a.astype(np.float16).T
    tA[TA_ONE] = 1.0
    tB = np.zeros((128, ns), np.float16)
    tB[TB_B:TB_ONE, :bc] = b.astype(np.float16).T
    tB[TB_ONE] = 1.0
    m = meta.astype(np.float32)
    tB[TB_M:TB_END, :bc] = np.where(m > 0, m, m * 100.0).astype(np.float16).T
    tA[TA_A:TA_ONE, bc:] = tA[TA_A:TA_ONE, 0:1]
    tB[TB_B:TB_ONE, bc:] = tB[TB_B:TB_ONE, 0:1]
    tB[TB_M:TB_END, bc:] = tB[TB_M:TB_END, 0:1]
    return tA, tB


def build_bass(n_chunks):
    nc = bacc.Bacc(None, target_bir_lowering=False, debug=False)
    n_steps = n_chunks + PIPE                       # 84
    n_quads = (n_steps + 2 + QUAD - 1) // QUAD      # tiles 0..n_steps+1
    n_stream = n_quads * QUAD

    tA_d = nc.dram_tensor("tA", [128, n_stream * N], F16, kind="ExternalInput")
    tB_d = nc.dram_tensor("tB", [128, n_stream * N], F16, kind="ExternalInput")
    wt_d = nc.dram_tensor("wt", [128, 512], F16, kind="ExternalInput")
    y_d = nc.dram_tensor("y", [1, n_chunks * N], F16, kind="ExternalOutput")

    with tile.TileContext(nc) as tc:
        with (
            tc.tile_pool(name="const", bufs=1) as constp,
            tc.tile_pool(name="tap", bufs=6) as tap,
            tc.tile_pool(name="tbp", bufs=6) as tbp,
            tc.tile_pool(name="up", bufs=8) as up,
            tc.tile_pool(name="ps", bufs=2, space=bass.MemorySpace.PSUM) as ps,
        ):
            wt = constp.tile([128, 512], F16, tag="wt")
            nc.sync.dma_start(wt[:], wt_d[:])
            wg1, wg2 = wt[:, 0:128], wt[:, 128:256]
            wg3, wg4 = wt[:, 256:384], wt[:, 384:512]

            def chain(*insts):
                for i in range(1, len(insts)):
                    add_dep_helper(insts[i].ins, insts[i - 1].ins,
                                   sync=False, reason="psum acc order")

            TAq, TBq, U = {}, {}, {}

            def new_quad(p):
                if p >= n_quads:
                    return
                TAq[p] = tap.tile([128, QUAD * N], F16, tag="ta", name=f"ta_{p}")
                TBq[p] = tbp.tile([128, QUAD * N], F16, tag="tb", name=f"tb_{p}")
                qs = slice(p * QUAD * N, (p + 1) * QUAD * N)
                nc.sync.dma_start(TAq[p][0:128], tA_d[:, qs])
                nc.sync.dma_start(TBq[p][0:128], tB_d[:, qs])

            def new_u(t):
                U[t] = up.tile([128, 2 * N], F16, tag="u", name=f"u_{t}")
                if t <= 1:
                    nc.gpsimd.memset(U[t][0:128], 0.0)

            new_quad(0)
            new_quad(1)
            new_u(0)
            new_u(1)

            mm = nc.tensor.matmul
            for t in range(n_steps):
                if (t + 6) % QUAD == 0:
                    new_quad((t + 6) // QUAD)
                new_u(t + 2)
                ta = TAq[t // QUAD]
                tb = TBq[t // QUAD]
                cs = slice((t % QUAD) * N, (t % QUAD + 1) * N)
                dt = t + 2                           # drain target step
                dta = TAq[dt // QUAD]
                dtb = TBq[dt // QUAD]
                dcs = slice((dt % QUAD) * N, (dt % QUAD + 1) * N)

                qab = ps.tile([128, 2 * N], F32, tag="qab", name=f"qab_{t}")
                mm(qab[0:128, 0:N], wg1[0:TA_END], ta[0:TA_END, cs],
                   start=True, stop=True, tile_position=(0, 0))
                mm(qab[0:128, N:2 * N], wg2[0:TB_END], tb[0:TB_END, cs],
                   start=True, stop=True, tile_position=(0, 0))
                q34 = ps.tile([128, N], F32, tag="q34", name=f"q34_{t}")
                i3 = mm(q34[0:128], wg3[0:TC_END], U[t][0:TC_END, 0:N],
                        start=True, stop=False, tile_position=(0, 0))
                i4 = mm(q34[0:128], wg4[0:TD_END], U[t][0:TD_END, N:2 * N],
                        start=False, stop=True, tile_position=(0, 0))
                chain(i3, i4)

                # ---- drains into step t+2 tiles ----
                nc.scalar.activation(U[t + 2][0:128, 0:2 * N],
                                     qab[0:128, 0:2 * N], PRELU, alpha=ALPHA)
                nc.scalar.activation(dta[TA_C0:TA_CE, dcs],
                                     q34[Q34_C0:Q34_CE], PRELU, alpha=ALPHA)
                nc.vector.tensor_copy(dtb[TB_C4:TB_B, dcs],
                                      q34[Q34_C4:Q34_C4E])
                nc.vector.scalar_tensor_tensor(
                    dtb[TB_C4:TB_B, dcs], dtb[TB_C4:TB_B, dcs],
                    ALPHA, dtb[TB_C4:TB_B, dcs], ALU.mult, ALU.max)

                # ---- y out (chunk t-20 sits in U[t+2] row 127, TC half) ----
                if t >= PIPE:
                    c = t - PIPE
                    nc.gpsimd.dma_start(y_d[:, c * N:(c + 1) * N],
                                        U[t + 2][TC_Y:TC_END, 0:N])

    nc.compile()
    return nc


_NC_CACHE = {}


def _get_nc(n_chunks):
    if n_chunks not in _NC_CACHE:
        _NC_CACHE[n_chunks] = build_bass(n_chunks)
    return _NC_CACHE[n_chunks]


def run_cores(inputs, n_chunks, cores, trace=False, trace_kwargs=None):
    a = np.asarray(inputs["a"], np.float32)
    b = np.asarray(inputs["b"], np.float32)
    meta = np.asarray(inputs["meta"], np.float32)
    wt = _pack_weights(inputs)
    n_steps = n_chunks + PIPE
    n_stream = ((n_steps + 2 + QUAD - 1) // QUAD) * QUAD
    in_maps = []
    for r in cores:
        sl = slice(r * B_CORE, r * B_CORE + n_chunks * N)
        tA, tB = _pack_core_inputs(a[sl], b[sl], meta[sl], n_stream)
        in_maps.append({"tA": tA, "tB": tB, "wt": wt})
    nc = _get_nc(n_chunks)
    kw = dict(trace=trace)
    if trace_kwargs:
        kw.update(trace_kwargs)
    res = run_bass_kernel_spmd(nc, in_maps, list(range(len(cores))), **kw)
    return [res.results[i]["y"] for i in range(len(cores))], res


def kernel(**inputs):
    n_chunks = B_CORE // N
    ys, _ = run_cores(inputs, n_chunks, list(range(N_CORES)))
    out = np.empty((B_FULL, 1), np.float32)
    for r in range(N_CORES):
        out[r * B_CORE:(r + 1) * B_CORE, 0] = ys[r][0].astype(np.float32)
    return out
